# revision 11
# baseline (speedup 1.0000x reference)
"""3-layer edge-gated GCN (PyG GCNConv-style) on 8 TRN2 NeuronCores.

v2 strategy (self-contained, shapes hardcoded for N=50000, E=800000, D=256):
  - Shard nodes 8 ways (6250/core, padded to 6272 = 49*128 rows).
  - Algebra: deg[v] = sum_{dst=v} ew + 1, dinv = deg^-1/2,
      h' = (x @ W + b) * dinv[:, None]
      out = relu?( dinv * (SUM_{e: dst=v} ew_e * h'[src_e] + h'[v]) )
  - x kept TRANSPOSED in SBUF (xT[din, row]); layer output re-transposed via
    SBUF->SBUF XBAR DMA transpose. No TensorE transposes, no PSUM->SBUF
    copies for matmul inputs. Bias folded into the matmul as a K=1 rank.
  - Per layer: matmul -> two AllGathers (tiles 0..23 -> table A, rest -> B,
    bf16) -> chunked dma_gather of source rows (3 dst-tiles per call) ->
    scatter matrices built ON THE FLY per 128-edge block on VectorE
    (iota is_equal dcol) * ew -- no 32MB/layer scatter-matrix DRAM traffic
    -> TensorE segment-sum matmuls -> epilogue on Scalar engine.
  - Edges partitioned by dst owner, dst tile (49/core), and src table half;
    per-(tile,group) block counts padded to the max over cores so all 8
    cores run the identical program (SPMD). Gather padding slots use idx 0
    and dcol -1 (zero scatter column); per-call trailing pad is trimmed via
    num_idxs_reg.
"""
import os
import sys
sys.path.insert(0, "/opt/trn_rl_repo")

import numpy as np
import ml_dtypes

import concourse.bass as bass
import concourse.tile as tile
from concourse import bacc, mybir
from concourse.bass_utils import run_bass_kernel_spmd

F32 = mybir.dt.float32
BF16 = mybir.dt.bfloat16
I16 = mybir.dt.int16
I32 = mybir.dt.int32
AL = mybir.AluOpType
AF = mybir.ActivationFunctionType

N, E, D = 50000, 800000, 256
C = 8                  # cores
SH = N // C            # 6250 real rows per shard
T = 49                 # dst tiles per core
SHP = T * 128          # 6272 padded rows per shard
TA = 24                # tiles 0..23 -> collective A
TB = T - TA            # tiles 24..48 -> collective B
RA = TA * 128          # 3072 rows per core in table A
RB = TB * 128          # 3200 rows per core in table B
NRA = C * RA           # 24576 rows in table A (fits int16)
NRB = C * RB           # 25600 rows in table B (fits int16)
CH = 3                 # dst tiles per gather call


def _chunks():
    out = []
    t0 = 0
    while t0 < T:
        out.append(list(range(t0, min(t0 + CH, T))))
        t0 += CH
    return out


def _host_prep(x, edge_index, edge_attr):
    """Pure index/layout preprocessing (no float math on values)."""
    src = np.asarray(edge_index[0], dtype=np.int64)
    dst = np.asarray(edge_index[1], dtype=np.int64)
    attr = np.asarray(edge_attr, dtype=np.float32).reshape(-1)

    owner_d = dst // SH
    dl = dst - owner_d * SH               # 0..6249
    tl = dl // 128                        # dst tile 0..48
    dcol = (dl % 128).astype(np.float32)
    owner_s = src // SH
    sl = src - owner_s * SH               # local src row 0..6249
    g = (sl >= RA).astype(np.int64)       # 0 = table A, 1 = table B
    rowA = owner_s * RA + sl
    rowB = owner_s * RB + (sl - RA)
    grow = np.where(g == 1, rowB, rowA)

    seg = (owner_d * T + tl) * 2 + g      # per (core, tile, group)
    order = np.argsort(seg, kind="stable")
    seg_sorted = seg[order]
    counts_e = np.bincount(seg_sorted, minlength=C * T * 2)
    seg_starts = np.concatenate([[0], np.cumsum(counts_e)[:-1]])
    rank_in_seg = np.arange(E) - seg_starts[seg_sorted]

    cnt_ctg = counts_e.reshape(C, T, 2)
    Bg = np.maximum((cnt_ctg.max(axis=0) + 127) // 128, 1)   # [T, 2]
    BA = Bg[:, 0].astype(int)
    BB = Bg[:, 1].astype(int)

    chunks = _chunks()
    NCH = len(chunks)
    # global block layout: for ch: for g in (A,B): for t in ch: B_g(t) blocks
    block_base = {}
    call_meta = []            # per (ch, g): (base_blk, nblk)
    nb = 0
    for ci, ch in enumerate(chunks):
        for gg in range(2):
            base = nb
            for t in ch:
                block_base[(t, gg)] = nb
                nb += int(Bg[t, gg])
            call_meta.append((base, nb - base))
    NBLK = nb
    e_pad = NBLK * 128

    # slot index for every edge
    sbase_tg = np.zeros((T, 2), np.int64)
    for t in range(T):
        for gg in range(2):
            sbase_tg[t, gg] = block_base[(t, gg)] * 128
    slot_sorted = sbase_tg[tl[order], g[order]] + rank_in_seg
    core_sorted = owner_d[order]

    # Pad slots gather row 0 of their table (valid, finite) and carry
    # dcol=-1 so their scatter-matrix column is all-zero. All slots are
    # fetched (num_idxs_reg == num_idxs), so no stale-SBUF reads anywhere.
    gidx_all = np.zeros((C, e_pad), dtype=np.int16)
    dcol_all = np.full((C, e_pad), -1.0, dtype=np.float32)
    attr_all = np.zeros((C, e_pad), dtype=np.float32)
    gidx_all[core_sorted, slot_sorted] = grow[order].astype(np.int16)
    dcol_all[core_sorted, slot_sorted] = dcol[order]
    attr_all[core_sorted, slot_sorted] = attr[order]

    i = np.arange(e_pad)
    idx_tiles = []
    for c in range(C):
        t16 = np.zeros((16, e_pad // 16), dtype=np.int16)
        t16[i % 16, i // 16] = gidx_all[c]
        idx_tiles.append(np.tile(t16, (8, 1)))

    dcol_tm = [np.ascontiguousarray(dcol_all[c].reshape(-1, 128).T) for c in range(C)]
    attr_tm = [np.ascontiguousarray(attr_all[c].reshape(-1, 128).T) for c in range(C)]

    # transposed x per shard: xT[p, t, j, r] = x[t*128+r, j*128+p]
    xf = np.asarray(x, dtype=np.float32)
    xts = []
    for c in range(C):
        pad = np.zeros((SHP, D), dtype=np.float32)
        pad[:SH] = xf[c * SH:(c + 1) * SH]
        v = pad.reshape(T, 128, 2, 128)           # [t, r, j, p]
        xts.append(np.ascontiguousarray(v.transpose(3, 0, 2, 1)).reshape(128, T * 2 * 128))

    meta = {
        "BA": tuple(int(b) for b in BA),
        "BB": tuple(int(b) for b in BB),
        "chunks": chunks,
        "call_meta": call_meta,
        "block_base": block_base,
        "NBLK": NBLK,
    }
    return xts, idx_tiles, dcol_tm, attr_tm, meta


def _build(meta):
    BA, BB = meta["BA"], meta["BB"]
    chunks = meta["chunks"]
    call_meta = meta["call_meta"]
    block_base = meta["block_base"]
    NBLK = meta["NBLK"]
    NCH = len(chunks)
    e_pad = NBLK * 128
    NBA_MAX = max(call_meta[2 * ci][1] for ci in range(NCH))
    NBB_MAX = max(call_meta[2 * ci + 1][1] for ci in range(NCH))

    nc = bacc.Bacc("TRN2", target_bir_lowering=False, debug=False,
                   num_devices=C, num_swdge_queues=4)

    xt_d = nc.declare_dram_parameter("xt", [128, T * 2 * 128], F32, isOutput=False)
    idx_d = nc.declare_dram_parameter("idx", [128, e_pad // 16], I16, isOutput=False)
    dcol_d = nc.declare_dram_parameter("dcol", [128, NBLK], F32, isOutput=False)
    attr_d = nc.declare_dram_parameter("attr", [128, NBLK], F32, isOutput=False)
    iota_d = nc.declare_dram_parameter("iota", [128, 128], BF16, isOutput=False)
    W_d = [nc.declare_dram_parameter(f"W{l+1}", [128, 2 * D], F32, isOutput=False)
           for l in range(3)]
    b_d = [nc.declare_dram_parameter(f"b{l+1}", [1, D], F32, isOutput=False)
           for l in range(3)]
    mw1_d = nc.declare_dram_parameter("mw1", [128, 8], F32, isOutput=False)
    mb1_d = nc.declare_dram_parameter("mb1", [128, 8], F32, isOutput=False)
    mw2_d = nc.declare_dram_parameter("mw2", [128, 8], F32, isOutput=False)
    mb2_d = nc.declare_dram_parameter("mb2", [128, 1], F32, isOutput=False)
    out_d = nc.declare_dram_parameter("out", [128, T, D], F32, isOutput=True)

    agiA = [nc.dram_tensor(f"agiA{l}", [RA, D], BF16) for l in range(3)]
    agiB = [nc.dram_tensor(f"agiB{l}", [RB, D], BF16) for l in range(3)]
    agoA = [nc.dram_tensor(f"agoA{l}", [NRA, D], BF16, addr_space="Shared")
            for l in range(3)]
    agoB = [nc.dram_tensor(f"agoB{l}", [NRB, D], BF16, addr_space="Shared")
            for l in range(3)]

    import contextlib
    rstack = contextlib.ExitStack()
    with tile.TileContext(nc) as tc:
        with (
            tc.tile_pool(name="res", bufs=1) as res,
            tc.tile_pool(name="work", bufs=2) as work,
            tc.tile_pool(name="spool", bufs=8) as spool,
            tc.tile_pool(name="gpoolA", bufs=2) as gpoolA,
            tc.tile_pool(name="gpoolB", bufs=2) as gpoolB,
            tc.tile_pool(name="php", bufs=2, space="PSUM") as php,
            tc.tile_pool(name="pdp", bufs=2, space="PSUM") as pdp,
            tc.tile_pool(name="pagg", bufs=3, space="PSUM") as pagg,
        ):
            xT = res.tile([128, T, 2, 128], BF16, tag="xT")
            hb16 = res.tile([128, T, D], BF16, tag="hb16")
            aggA = res.tile([128, T, D], BF16, tag="aggA")
            idx_r = res.tile([128, e_pad // 16], I16, tag="idx")
            dcol_r = res.tile([128, NBLK], F32, tag="dcol")
            ew_r = res.tile([128, NBLK], F32, tag="ew")
            iota_r = res.tile([128, 128], BF16, tag="iota")
            dinv_r = res.tile([128, T], F32, tag="dinv")
            Wt = [res.tile([128, 2, D], BF16, name=f"Wt{l}", tag=f"W{l}") for l in range(3)]
            brow = [res.tile([1, D], BF16, name=f"brow{l}", tag=f"b{l}") for l in range(3)]
            ones1 = res.tile([1, 128], BF16, tag="ones1")
            onesc = res.tile([128, 1], BF16, tag="onesc")
            mw1_r = res.tile([128, 8], F32, tag="mw1")
            mb1_r = res.tile([128, 8], F32, tag="mb1")
            mw2_r = res.tile([128, 8], F32, tag="mw2")
            mb2_r = res.tile([128, 1], F32, tag="mb2")

            nc.sync.dma_start(idx_r[:], idx_d.ap())
            nc.sync.dma_start(dcol_r[:], dcol_d.ap())
            nc.sync.dma_start(iota_r[:], iota_d.ap())
            nc.sync.dma_start(mw1_r[:], mw1_d.ap())
            nc.sync.dma_start(mb1_r[:], mb1_d.ap())
            nc.sync.dma_start(mw2_r[:], mw2_d.ap())
            nc.sync.dma_start(mb2_r[:], mb2_d.ap())
            nc.gpsimd.dma_start(
                xT[:], xt_d.ap().rearrange("p (t k r) -> p t k r", t=T, k=2))
            for l in range(3):
                nc.gpsimd.dma_start(
                    Wt[l][:], W_d[l].ap().rearrange("p (k o) -> p k o", k=2))
                nc.gpsimd.dma_start(brow[l][:], b_d[l].ap())
            nc.gpsimd.memset(ones1[:], 1.0)
            nc.gpsimd.memset(onesc[:], 1.0)

            # ---- edge MLP: ew = sigmoid(mw2 @ relu(attr*mw1 + mb1) + mb2) ----
            attr_r = work.tile([128, NBLK], F32, tag="attr", bufs=1)
            nc.sync.dma_start(attr_r[:], attr_d.ap())
            acc = None
            for j in range(8):
                tj = work.tile([128, NBLK], F32, tag="mlptmp", bufs=2, name=f"tj{j}")
                nc.scalar.activation(tj[:], attr_r[:], AF.Relu,
                                     bias=mb1_r[:, j:j + 1], scale=mw1_r[:, j:j + 1])
                nacc = work.tile([128, NBLK], F32, tag="mlpacc", bufs=2, name=f"acc{j}")
                if j == 0:
                    nc.vector.tensor_scalar_mul(nacc[:], tj[:], mw2_r[:, j:j + 1])
                else:
                    nc.vector.scalar_tensor_tensor(
                        nacc[:], tj[:], mw2_r[:, j:j + 1], acc[:],
                        op0=AL.mult, op1=AL.add)
                acc = nacc
            nc.scalar.activation(ew_r[:], acc[:], AF.Sigmoid, bias=mb2_r[:, 0:1])

            def build_s(gb, name):
                s = spool.tile([128, 128], BF16, tag="s", name=name)
                nc.vector.tensor_scalar(
                    s[:], iota_r[:], dcol_r[:, gb:gb + 1], ew_r[:, gb:gb + 1],
                    op0=AL.is_equal, op1=AL.mult)
                return s

            # ---- degree pass ----
            for t in range(T):
                blks = ([block_base[(t, 0)] + k for k in range(BA[t])]
                        + [block_base[(t, 1)] + k for k in range(BB[t])])
                dp = pdp.tile([128, 1], F32, tag="degp", name=f"dp{t}")
                for k, gb in enumerate(blks):
                    s = build_s(gb, f"sdeg_{t}_{k}")
                    nc.tensor.matmul(dp[:], s[:], onesc[:],
                                     start=(k == 0), stop=(k == len(blks) - 1))
                degs = work.tile([128, 1], F32, tag="degs", name=f"degs{t}")
                nc.vector.tensor_scalar_add(degs[:], dp[:], 1.0)
                rec = work.tile([128, 1], F32, tag="rec", name=f"rec{t}")
                nc.vector.reciprocal(rec[:], degs[:])
                nc.scalar.sqrt(dinv_r[:, t:t + 1], rec[:])

            # ---- layers ----
            for l in range(3):
                # phase A: h' = (x @ W + b) * dinv, write to agi table
                for t in range(T):
                    hp = php.tile([128, D], F32, tag="hp", name=f"hp{l}_{t}")
                    nc.tensor.matmul(hp[:], xT[:, t, 0, :], Wt[l][:, 0, :],
                                     start=True, stop=False)
                    nc.tensor.matmul(hp[:], xT[:, t, 1, :], Wt[l][:, 1, :],
                                     start=False, stop=False)
                    nc.tensor.matmul(hp[:], ones1[:], brow[l][:],
                                     start=False, stop=True)
                    nc.scalar.activation(hb16[:, t, :], hp[:], AF.Copy,
                                         bias=0.0, scale=dinv_r[:, t:t + 1])
                    if t < TA:
                        nc.sync.dma_start(agiA[l].ap()[t * 128:(t + 1) * 128],
                                          hb16[:, t, :])
                    else:
                        nc.sync.dma_start(agiB[l].ap()[(t - TA) * 128:(t - TA + 1) * 128],
                                          hb16[:, t, :])
                    if t == TA - 1:
                        nc.gpsimd.collective_compute(
                            "AllGather", AL.bypass,
                            replica_groups=[list(range(C))],
                            ins=[agiA[l].ap().opt()],
                            outs=[agoA[l].ap().opt()],
                        )
                nc.gpsimd.collective_compute(
                    "AllGather", AL.bypass,
                    replica_groups=[list(range(C))],
                    ins=[agiB[l].ap().opt()],
                    outs=[agoB[l].ap().opt()],
                )

                # A sweep: partial aggregation from table A
                for ci, ch in enumerate(chunks):
                    base_blk, nblk = call_meta[2 * ci]
                    ga = gpoolA.tile([128, NBA_MAX, D], BF16, tag="gA",
                                     name=f"gA_{l}_{ci}")
                    nc.gpsimd.dma_gather(
                        ga[:, 0:nblk, :], agoA[l].ap(),
                        idx_r[:, base_blk * 8:(base_blk + nblk) * 8],
                        num_idxs=nblk * 128, num_idxs_reg=nblk * 128,
                        elem_size=D, single_packet=False,
                        queue_num=0)
                    for t in ch:
                        nba = BA[t]
                        loc0 = block_base[(t, 0)] - base_blk
                        ap_ = pagg.tile([128, D], F32, tag="aggp",
                                        name=f"apA_{l}_{t}")
                        for k in range(nba):
                            s = build_s(block_base[(t, 0)] + k, f"sA_{l}_{t}_{k}")
                            nc.tensor.matmul(ap_[:], s[:], ga[:, loc0 + k, :],
                                             start=(k == 0), stop=(k == nba - 1))
                        nc.scalar.activation(aggA[:, t, :], ap_[:], AF.Copy)

                # B sweep: finish aggregation, epilogue
                for ci, ch in enumerate(chunks):
                    base_blk, nblk = call_meta[2 * ci + 1]
                    gb_ = gpoolB.tile([128, NBB_MAX, D], BF16, tag="gB",
                                      name=f"gB_{l}_{ci}")
                    nc.gpsimd.dma_gather(
                        gb_[:, 0:nblk, :], agoB[l].ap(),
                        idx_r[:, base_blk * 8:(base_blk + nblk) * 8],
                        num_idxs=nblk * 128, num_idxs_reg=nblk * 128,
                        elem_size=D, single_packet=False,
                        queue_num=0)
                    for t in ch:
                        nbb = BB[t]
                        loc0 = block_base[(t, 1)] - base_blk
                        ap_ = pagg.tile([128, D], F32, tag="aggp",
                                        name=f"apB_{l}_{t}")
                        for k in range(nbb):
                            s = build_s(block_base[(t, 1)] + k, f"sB_{l}_{t}_{k}")
                            nc.tensor.matmul(ap_[:], s[:], gb_[:, loc0 + k, :],
                                             start=(k == 0), stop=(k == nbb - 1))
                        tmp = work.tile([128, D], F32, tag="ep1", bufs=3,
                                        name=f"t1_{l}_{t}")
                        nc.vector.tensor_add(tmp[:], ap_[:], aggA[:, t, :])
                        tmp2 = work.tile([128, D], F32, tag="ep2", bufs=3,
                                         name=f"t2_{l}_{t}")
                        nc.vector.tensor_add(tmp2[:], tmp[:], hb16[:, t, :])
                        if l < 2:
                            ob = work.tile([128, D], BF16, tag="obh", bufs=3,
                                           name=f"ob_{l}_{t}")
                            nc.scalar.activation(ob[:], tmp2[:], AF.Relu,
                                                 scale=dinv_r[:, t:t + 1])
                            nc.sync.dma_start(xT[:, t, :, :], ob[:], transpose=True)
                        else:
                            ob = work.tile([128, D], F32, tag="obf", bufs=3,
                                           name=f"obf_{t}")
                            nc.scalar.activation(ob[:], tmp2[:], AF.Copy,
                                                 scale=dinv_r[:, t:t + 1])
                            nc.sync.dma_start(out_d.ap()[:, t, :], ob[:])

    nc.compile()
    return nc


_CACHE = {}


def kernel(x, edge_index, edge_attr, W1, b1, W2, b2, W3, b3, mw1, mb1, mw2, mb2):
    xts, idx_tiles, dcol_tm, attr_tm, meta = _host_prep(x, edge_index, edge_attr)

    key = (meta["BA"], meta["BB"])
    if key not in _CACHE:
        _CACHE[key] = _build(meta)
    nc = _CACHE[key]

    iota = np.tile(np.arange(128, dtype=np.float32)[None, :], (128, 1)).astype(
        ml_dtypes.bfloat16)
    mw1_b = np.tile(np.asarray(mw1, np.float32).reshape(1, 8), (128, 1))
    mb1_b = np.tile(np.asarray(mb1, np.float32).reshape(1, 8), (128, 1))
    mw2_b = np.tile(np.asarray(mw2, np.float32).reshape(1, 8), (128, 1))
    mb2_b = np.tile(np.asarray(mb2, np.float32).reshape(1, 1), (128, 1))
    Ws = []
    for w in (W1, W2, W3):
        wf = np.asarray(w, np.float32)            # [256, 256]
        wt = wf.reshape(2, 128, D).transpose(1, 0, 2).reshape(128, 2 * D)
        Ws.append(np.ascontiguousarray(wt))
    bs = [np.asarray(b, np.float32).reshape(1, D) for b in (b1, b2, b3)]

    in_maps = []
    for c in range(C):
        in_maps.append({
            "xt": xts[c], "idx": idx_tiles[c], "dcol": dcol_tm[c],
            "attr": attr_tm[c], "iota": iota,
            "W1": Ws[0], "W2": Ws[1], "W3": Ws[2],
            "b1": bs[0], "b2": bs[1], "b3": bs[2],
            "mw1": mw1_b, "mb1": mb1_b, "mw2": mw2_b, "mb2": mb2_b,
        })
    res = run_bass_kernel_spmd(nc, in_maps, core_ids=list(range(C)))
    kernel.last_result = res
    outs = []
    for c in range(C):
        o = res.results[c]["out"]            # [128, T, D]
        rows = o.transpose(1, 0, 2).reshape(SHP, D)[:SH]
        outs.append(rows)
    return np.concatenate(outs, axis=0).astype(np.float32)


# revision 13
# speedup vs baseline: 1.4630x; 1.4630x over previous
"""3-layer edge-gated GCN (PyG GCNConv-style) on 8 TRN2 NeuronCores.

v2 strategy (self-contained, shapes hardcoded for N=50000, E=800000, D=256):
  - Shard nodes 8 ways (6250/core, padded to 6272 = 49*128 rows).
  - Algebra: deg[v] = sum_{dst=v} ew + 1, dinv = deg^-1/2,
      h' = (x @ W + b) * dinv[:, None]
      out = relu?( dinv * (SUM_{e: dst=v} ew_e * h'[src_e] + h'[v]) )
  - x kept TRANSPOSED in SBUF (xT[din, row]); layer output re-transposed via
    SBUF->SBUF XBAR DMA transpose. No TensorE transposes, no PSUM->SBUF
    copies for matmul inputs. Bias folded into the matmul as a K=1 rank.
  - Per layer: matmul -> two AllGathers (tiles 0..23 -> table A, rest -> B,
    bf16) -> chunked dma_gather of source rows (3 dst-tiles per call) ->
    scatter matrices built ON THE FLY per 128-edge block on VectorE
    (iota is_equal dcol) * ew -- no 32MB/layer scatter-matrix DRAM traffic
    -> TensorE segment-sum matmuls -> epilogue on Scalar engine.
  - Edges partitioned by dst owner, dst tile (49/core), and src table half;
    per-(tile,group) block counts padded to the max over cores so all 8
    cores run the identical program (SPMD). Gather padding slots use idx 0
    and dcol -1 (zero scatter column); per-call trailing pad is trimmed via
    num_idxs_reg.
"""
import os
import sys
sys.path.insert(0, "/opt/trn_rl_repo")

import numpy as np
import ml_dtypes

import concourse.bass as bass
import concourse.tile as tile
from concourse import bacc, mybir
from concourse.bass_utils import run_bass_kernel_spmd

F32 = mybir.dt.float32
BF16 = mybir.dt.bfloat16
I16 = mybir.dt.int16
I32 = mybir.dt.int32
AL = mybir.AluOpType
AF = mybir.ActivationFunctionType

N, E, D = 50000, 800000, 256
C = 8                  # cores
SH = N // C            # 6250 real rows per shard
T = 49                 # dst tiles per core
SHP = T * 128          # 6272 padded rows per shard
TA = 24                # tiles 0..23 -> collective A
TB = T - TA            # tiles 24..48 -> collective B
RA = TA * 128          # 3072 rows per core in table A
RB = TB * 128          # 3200 rows per core in table B
NRA = C * RA           # 24576 rows in table A (fits int16)
NRB = C * RB           # 25600 rows in table B (fits int16)
CH = 1                 # dst tiles per gather call


def _chunks():
    out = []
    t0 = 0
    while t0 < T:
        out.append(list(range(t0, min(t0 + CH, T))))
        t0 += CH
    return out


def _host_prep(x, edge_index, edge_attr):
    """Pure index/layout preprocessing (no float math on values)."""
    src = np.asarray(edge_index[0], dtype=np.int64)
    dst = np.asarray(edge_index[1], dtype=np.int64)
    attr = np.asarray(edge_attr, dtype=np.float32).reshape(-1)

    owner_d = dst // SH
    dl = dst - owner_d * SH               # 0..6249
    tl = dl // 128                        # dst tile 0..48
    dcol = (dl % 128).astype(np.float32)
    owner_s = src // SH
    sl = src - owner_s * SH               # local src row 0..6249
    g = (sl >= RA).astype(np.int64)       # 0 = table A, 1 = table B
    rowA = owner_s * RA + sl
    rowB = owner_s * RB + (sl - RA)
    grow = np.where(g == 1, rowB, rowA)

    seg = (owner_d * T + tl) * 2 + g      # per (core, tile, group)
    order = np.argsort(seg, kind="stable")
    seg_sorted = seg[order]
    counts_e = np.bincount(seg_sorted, minlength=C * T * 2)
    seg_starts = np.concatenate([[0], np.cumsum(counts_e)[:-1]])
    rank_in_seg = np.arange(E) - seg_starts[seg_sorted]

    cnt_ctg = counts_e.reshape(C, T, 2)
    Bg = np.maximum((cnt_ctg.max(axis=0) + 127) // 128, 1)   # [T, 2]
    BA = Bg[:, 0].astype(int)
    BB = Bg[:, 1].astype(int)

    chunks = _chunks()
    NCH = len(chunks)
    # global block layout: for ch: for g in (A,B): for t in ch: B_g(t) blocks
    block_base = {}
    call_meta = []            # per (ch, g): (base_blk, nblk)
    nb = 0
    for ci, ch in enumerate(chunks):
        for gg in range(2):
            base = nb
            for t in ch:
                block_base[(t, gg)] = nb
                nb += int(Bg[t, gg])
            call_meta.append((base, nb - base))
    NBLK = nb
    e_pad = NBLK * 128

    # slot index for every edge
    sbase_tg = np.zeros((T, 2), np.int64)
    for t in range(T):
        for gg in range(2):
            sbase_tg[t, gg] = block_base[(t, gg)] * 128
    slot_sorted = sbase_tg[tl[order], g[order]] + rank_in_seg
    core_sorted = owner_d[order]

    # Pad slots gather row 0 of their table (valid, finite) and carry
    # dcol=-1 so their scatter-matrix column is all-zero. All slots are
    # fetched (num_idxs_reg == num_idxs), so no stale-SBUF reads anywhere.
    gidx_all = np.zeros((C, e_pad), dtype=np.int16)
    dcol_all = np.full((C, e_pad), -1.0, dtype=np.float32)
    attr_all = np.zeros((C, e_pad), dtype=np.float32)
    gidx_all[core_sorted, slot_sorted] = grow[order].astype(np.int16)
    dcol_all[core_sorted, slot_sorted] = dcol[order]
    attr_all[core_sorted, slot_sorted] = attr[order]

    i = np.arange(e_pad)
    idx_tiles = []
    for c in range(C):
        t16 = np.zeros((16, e_pad // 16), dtype=np.int16)
        t16[i % 16, i // 16] = gidx_all[c]
        idx_tiles.append(np.tile(t16, (8, 1)))

    dcol_tm = [np.ascontiguousarray(dcol_all[c].reshape(-1, 128).T) for c in range(C)]
    attr_tm = [np.ascontiguousarray(attr_all[c].reshape(-1, 128).T) for c in range(C)]

    # transposed x per shard: xT[p, t, j, r] = x[t*128+r, j*128+p]
    xf = np.asarray(x, dtype=np.float32)
    xts = []
    for c in range(C):
        pad = np.zeros((SHP, D), dtype=np.float32)
        pad[:SH] = xf[c * SH:(c + 1) * SH]
        v = pad.reshape(T, 128, 2, 128)           # [t, r, j, p]
        xts.append(np.ascontiguousarray(v.transpose(3, 0, 2, 1)).reshape(
            128, T * 2 * 128).astype(ml_dtypes.bfloat16))

    meta = {
        "BA": tuple(int(b) for b in BA),
        "BB": tuple(int(b) for b in BB),
        "chunks": chunks,
        "call_meta": call_meta,
        "block_base": block_base,
        "NBLK": NBLK,
    }
    return xts, idx_tiles, dcol_tm, attr_tm, meta


def _build(meta):
    BA, BB = meta["BA"], meta["BB"]
    chunks = meta["chunks"]
    call_meta = meta["call_meta"]
    block_base = meta["block_base"]
    NBLK = meta["NBLK"]
    NCH = len(chunks)
    e_pad = NBLK * 128
    NBA_MAX = max(call_meta[2 * ci][1] for ci in range(NCH))
    NBB_MAX = max(call_meta[2 * ci + 1][1] for ci in range(NCH))

    nc = bacc.Bacc("TRN2", target_bir_lowering=False, debug=False,
                   num_devices=C, num_swdge_queues=4)

    xt_d = nc.declare_dram_parameter("xt", [128, T * 2 * 128], BF16, isOutput=False)
    idx_d = nc.declare_dram_parameter("idx", [128, e_pad // 16], I16, isOutput=False)
    dcol_d = nc.declare_dram_parameter("dcol", [128, NBLK], F32, isOutput=False)
    attr_d = nc.declare_dram_parameter("attr", [128, NBLK], F32, isOutput=False)
    iota_d = nc.declare_dram_parameter("iota", [128, 128], BF16, isOutput=False)
    W_d = [nc.declare_dram_parameter(f"W{l+1}", [128, 2 * D], BF16, isOutput=False)
           for l in range(3)]
    b_d = [nc.declare_dram_parameter(f"b{l+1}", [1, D], BF16, isOutput=False)
           for l in range(3)]
    mw1_d = nc.declare_dram_parameter("mw1", [128, 8], F32, isOutput=False)
    mb1_d = nc.declare_dram_parameter("mb1", [128, 8], F32, isOutput=False)
    mw2_d = nc.declare_dram_parameter("mw2", [128, 8], F32, isOutput=False)
    mb2_d = nc.declare_dram_parameter("mb2", [128, 1], F32, isOutput=False)
    out_d = nc.declare_dram_parameter("out", [128, T, D], F32, isOutput=True)

    agiA = [nc.dram_tensor(f"agiA{l}", [RA, D], BF16) for l in range(3)]
    agiB = [nc.dram_tensor(f"agiB{l}", [RB, D], BF16) for l in range(3)]
    agoA = [nc.dram_tensor(f"agoA{l}", [NRA, D], BF16, addr_space="Shared")
            for l in range(3)]
    agoB = [nc.dram_tensor(f"agoB{l}", [NRB, D], BF16, addr_space="Shared")
            for l in range(3)]

    import contextlib
    rstack = contextlib.ExitStack()
    with tile.TileContext(nc) as tc:
        with (
            tc.tile_pool(name="res", bufs=1) as res,
            tc.tile_pool(name="work", bufs=2) as work,
            tc.tile_pool(name="spool", bufs=12) as spool,
            tc.tile_pool(name="gpoolA", bufs=4) as gpoolA,
            tc.tile_pool(name="gpoolB", bufs=4) as gpoolB,
            tc.tile_pool(name="php", bufs=2, space="PSUM") as php,
            tc.tile_pool(name="pdp", bufs=2, space="PSUM") as pdp,
            tc.tile_pool(name="pagg", bufs=3, space="PSUM") as pagg,
        ):
            xT = res.tile([128, T, 2, 128], BF16, tag="xT")
            hb16 = res.tile([128, T, D], BF16, tag="hb16")
            aggA = res.tile([128, T, D], BF16, tag="aggA")
            idx_r = res.tile([128, e_pad // 16], I16, tag="idx")
            dcol_r = res.tile([128, NBLK], F32, tag="dcol")
            ew_r = res.tile([128, NBLK], F32, tag="ew")
            iota_r = res.tile([128, 128], BF16, tag="iota")
            dinv_r = res.tile([128, T], F32, tag="dinv")
            Wt = [res.tile([128, 2, D], BF16, name=f"Wt{l}", tag=f"W{l}") for l in range(3)]
            brow = [res.tile([1, D], BF16, name=f"brow{l}", tag=f"b{l}") for l in range(3)]
            ones1 = res.tile([1, 128], BF16, tag="ones1")
            onesc = res.tile([128, 1], BF16, tag="onesc")
            mw1_r = res.tile([128, 8], F32, tag="mw1")
            mb1_r = res.tile([128, 8], F32, tag="mb1")
            mw2_r = res.tile([128, 8], F32, tag="mw2")
            mb2_r = res.tile([128, 1], F32, tag="mb2")

            nc.sync.dma_start(idx_r[:], idx_d.ap())
            nc.sync.dma_start(dcol_r[:], dcol_d.ap())
            nc.sync.dma_start(iota_r[:], iota_d.ap())
            nc.sync.dma_start(mw1_r[:], mw1_d.ap())
            nc.sync.dma_start(mb1_r[:], mb1_d.ap())
            nc.sync.dma_start(mw2_r[:], mw2_d.ap())
            nc.sync.dma_start(mb2_r[:], mb2_d.ap())
            nc.sync.dma_start(
                xT[:], xt_d.ap().rearrange("p (t k r) -> p t k r", t=T, k=2))
            for l in range(3):
                nc.sync.dma_start(
                    Wt[l][:], W_d[l].ap().rearrange("p (k o) -> p k o", k=2))
                nc.sync.dma_start(brow[l][:], b_d[l].ap())
            nc.gpsimd.memset(ones1[:], 1.0)
            nc.gpsimd.memset(onesc[:], 1.0)

            # ---- edge MLP: ew = sigmoid(mw2 @ relu(attr*mw1 + mb1) + mb2) ----
            attr_r = work.tile([128, NBLK], F32, tag="attr", bufs=1)
            nc.sync.dma_start(attr_r[:], attr_d.ap())
            acc = None
            for j in range(8):
                tj = work.tile([128, NBLK], F32, tag="mlptmp", bufs=2, name=f"tj{j}")
                nc.scalar.activation(tj[:], attr_r[:], AF.Relu,
                                     bias=mb1_r[:, j:j + 1], scale=mw1_r[:, j:j + 1])
                nacc = work.tile([128, NBLK], F32, tag="mlpacc", bufs=2, name=f"acc{j}")
                if j == 0:
                    nc.vector.tensor_scalar_mul(nacc[:], tj[:], mw2_r[:, j:j + 1])
                else:
                    nc.vector.scalar_tensor_tensor(
                        nacc[:], tj[:], mw2_r[:, j:j + 1], acc[:],
                        op0=AL.mult, op1=AL.add)
                acc = nacc
            nc.scalar.activation(ew_r[:], acc[:], AF.Sigmoid, bias=mb2_r[:, 0:1])

            def build_s(gb, name):
                s = spool.tile([128, 128], BF16, tag="s", name=name)
                nc.vector.tensor_scalar(
                    s[:], iota_r[:], dcol_r[:, gb:gb + 1], ew_r[:, gb:gb + 1],
                    op0=AL.is_equal, op1=AL.mult)
                return s

            # ---- degree pass ----
            for t in range(T):
                blks = ([block_base[(t, 0)] + k for k in range(BA[t])]
                        + [block_base[(t, 1)] + k for k in range(BB[t])])
                dp = pdp.tile([128, 1], F32, tag="degp", name=f"dp{t}")
                for k, gb in enumerate(blks):
                    s = build_s(gb, f"sdeg_{t}_{k}")
                    nc.tensor.matmul(dp[:], s[:], onesc[:],
                                     start=(k == 0), stop=(k == len(blks) - 1))
                degs = work.tile([128, 1], F32, tag="degs", name=f"degs{t}")
                nc.vector.tensor_scalar_add(degs[:], dp[:], 1.0)
                rec = work.tile([128, 1], F32, tag="rec", name=f"rec{t}")
                nc.vector.reciprocal(rec[:], degs[:])
                nc.scalar.sqrt(dinv_r[:, t:t + 1], rec[:])

            # ---- layers ----
            qctr = [0]

            def next_q():
                q = qctr[0] % 4
                qctr[0] += 1
                return q

            for l in range(3):
                # phase A: h' = (x @ W + b) * dinv, write to agi table
                for t in range(T):
                    hp = php.tile([128, D], F32, tag="hp", name=f"hp{l}_{t}")
                    nc.tensor.matmul(hp[:], xT[:, t, 0, :], Wt[l][:, 0, :],
                                     start=True, stop=False)
                    nc.tensor.matmul(hp[:], xT[:, t, 1, :], Wt[l][:, 1, :],
                                     start=False, stop=False)
                    nc.tensor.matmul(hp[:], ones1[:], brow[l][:],
                                     start=False, stop=True)
                    nc.scalar.activation(hb16[:, t, :], hp[:], AF.Copy,
                                         bias=0.0, scale=dinv_r[:, t:t + 1])
                    if t < TA:
                        nc.sync.dma_start(agiA[l].ap()[t * 128:(t + 1) * 128],
                                          hb16[:, t, :])
                    else:
                        nc.sync.dma_start(agiB[l].ap()[(t - TA) * 128:(t - TA + 1) * 128],
                                          hb16[:, t, :])
                    if t == TA - 1:
                        nc.gpsimd.collective_compute(
                            "AllGather", AL.bypass,
                            replica_groups=[list(range(C))],
                            ins=[agiA[l].ap().opt()],
                            outs=[agoA[l].ap().opt()],
                        )
                nc.gpsimd.collective_compute(
                    "AllGather", AL.bypass,
                    replica_groups=[list(range(C))],
                    ins=[agiB[l].ap().opt()],
                    outs=[agoB[l].ap().opt()],
                )

                # A sweep: partial aggregation from table A
                for ci, ch in enumerate(chunks):
                    base_blk, nblk = call_meta[2 * ci]
                    ga = gpoolA.tile([128, NBA_MAX, D], BF16, tag="gA",
                                     name=f"gA_{l}_{ci}")
                    nc.gpsimd.dma_gather(
                        ga[:, 0:nblk, :], agoA[l].ap(),
                        idx_r[:, base_blk * 8:(base_blk + nblk) * 8],
                        num_idxs=nblk * 128, num_idxs_reg=nblk * 128,
                        elem_size=D, single_packet=False,
                        queue_num=next_q())
                    for t in ch:
                        nba = BA[t]
                        loc0 = block_base[(t, 0)] - base_blk
                        ap_ = pagg.tile([128, D], F32, tag="aggp",
                                        name=f"apA_{l}_{t}")
                        for k in range(nba):
                            s = build_s(block_base[(t, 0)] + k, f"sA_{l}_{t}_{k}")
                            nc.tensor.matmul(ap_[:], s[:], ga[:, loc0 + k, :],
                                             start=(k == 0), stop=(k == nba - 1))
                        nc.scalar.activation(aggA[:, t, :], ap_[:], AF.Copy)

                # B sweep: finish aggregation, epilogue
                for ci, ch in enumerate(chunks):
                    base_blk, nblk = call_meta[2 * ci + 1]
                    gb_ = gpoolB.tile([128, NBB_MAX, D], BF16, tag="gB",
                                      name=f"gB_{l}_{ci}")
                    nc.gpsimd.dma_gather(
                        gb_[:, 0:nblk, :], agoB[l].ap(),
                        idx_r[:, base_blk * 8:(base_blk + nblk) * 8],
                        num_idxs=nblk * 128, num_idxs_reg=nblk * 128,
                        elem_size=D, single_packet=False,
                        queue_num=next_q())
                    for t in ch:
                        nbb = BB[t]
                        loc0 = block_base[(t, 1)] - base_blk
                        ap_ = pagg.tile([128, D], F32, tag="aggp",
                                        name=f"apB_{l}_{t}")
                        for k in range(nbb):
                            s = build_s(block_base[(t, 1)] + k, f"sB_{l}_{t}_{k}")
                            nc.tensor.matmul(ap_[:], s[:], gb_[:, loc0 + k, :],
                                             start=(k == 0), stop=(k == nbb - 1))
                        tmp = work.tile([128, D], F32, tag="ep1", bufs=3,
                                        name=f"t1_{l}_{t}")
                        nc.vector.tensor_add(tmp[:], ap_[:], aggA[:, t, :])
                        tmp2 = work.tile([128, D], F32, tag="ep2", bufs=3,
                                         name=f"t2_{l}_{t}")
                        nc.vector.tensor_add(tmp2[:], tmp[:], hb16[:, t, :])
                        if l < 2:
                            ob = work.tile([128, D], BF16, tag="obh", bufs=3,
                                           name=f"ob_{l}_{t}")
                            nc.scalar.activation(ob[:], tmp2[:], AF.Relu,
                                                 scale=dinv_r[:, t:t + 1])
                            nc.sync.dma_start(xT[:, t, :, :], ob[:], transpose=True)
                        else:
                            ob = work.tile([128, D], F32, tag="obf", bufs=3,
                                           name=f"obf_{t}")
                            nc.scalar.activation(ob[:], tmp2[:], AF.Copy,
                                                 scale=dinv_r[:, t:t + 1])
                            nc.sync.dma_start(out_d.ap()[:, t, :], ob[:])

    nc.compile()
    return nc


_CACHE = {}


def kernel(x, edge_index, edge_attr, W1, b1, W2, b2, W3, b3, mw1, mb1, mw2, mb2):
    xts, idx_tiles, dcol_tm, attr_tm, meta = _host_prep(x, edge_index, edge_attr)

    key = (meta["BA"], meta["BB"])
    if key not in _CACHE:
        _CACHE[key] = _build(meta)
    nc = _CACHE[key]

    iota = np.tile(np.arange(128, dtype=np.float32)[None, :], (128, 1)).astype(
        ml_dtypes.bfloat16)
    mw1_b = np.tile(np.asarray(mw1, np.float32).reshape(1, 8), (128, 1))
    mb1_b = np.tile(np.asarray(mb1, np.float32).reshape(1, 8), (128, 1))
    mw2_b = np.tile(np.asarray(mw2, np.float32).reshape(1, 8), (128, 1))
    mb2_b = np.tile(np.asarray(mb2, np.float32).reshape(1, 1), (128, 1))
    Ws = []
    for w in (W1, W2, W3):
        wf = np.asarray(w, np.float32)            # [256, 256]
        wt = wf.reshape(2, 128, D).transpose(1, 0, 2).reshape(128, 2 * D)
        Ws.append(np.ascontiguousarray(wt).astype(ml_dtypes.bfloat16))
    bs = [np.asarray(b, np.float32).reshape(1, D).astype(ml_dtypes.bfloat16)
          for b in (b1, b2, b3)]

    in_maps = []
    for c in range(C):
        in_maps.append({
            "xt": xts[c], "idx": idx_tiles[c], "dcol": dcol_tm[c],
            "attr": attr_tm[c], "iota": iota,
            "W1": Ws[0], "W2": Ws[1], "W3": Ws[2],
            "b1": bs[0], "b2": bs[1], "b3": bs[2],
            "mw1": mw1_b, "mb1": mb1_b, "mw2": mw2_b, "mb2": mb2_b,
        })
    res = run_bass_kernel_spmd(nc, in_maps, core_ids=list(range(C)))
    kernel.last_result = res
    outs = []
    for c in range(C):
        o = res.results[c]["out"]            # [128, T, D]
        rows = o.transpose(1, 0, 2).reshape(SHP, D)[:SH]
        outs.append(rows)
    return np.concatenate(outs, axis=0).astype(np.float32)


# revision 14
# speedup vs baseline: 1.9675x; 1.3448x over previous
"""3-layer edge-gated GCN (PyG GCNConv-style) on 8 TRN2 NeuronCores.

Strategy (self-contained, shapes hardcoded for N=50000, E=800000, D=256):
  - Shard nodes 8 ways (6250/core, padded to 6272 = 49*128 rows).
  - Algebra: with deg[v] = sum_{dst=v} ew + 1, dinv = deg^-1/2,
      h'   = (x @ W + b) * dinv[:, None]
      out  = relu?( dinv * (SUM_{e: dst=v} ew_e * h'[src_e]  +  h'[v]) )
    which equals the reference GCN layer exactly (dinv[src] folded into h',
    dinv[dst] folded into the epilogue, self-loop = dinv^2 * h).
  - Per layer: local matmul -> AllGather h' (bf16) -> per-edge row gather
    (dma_gather, int16 idx, two 25088-row tables) -> segment-sum via TensorE
    matmuls against on-device-built one-hot*ew matrices -> fused epilogue.
  - Edges are partitioned by dst owner, grouped per 128-dst tile, split into
    lo/hi source-table halves, padded to a uniform block count so all 8 cores
    run the identical program (SPMD).
"""
import os
import sys
sys.path.insert(0, "/opt/trn_rl_repo")

import numpy as np
import ml_dtypes

import concourse.bass as bass
import concourse.tile as tile
from concourse import bacc, mybir
from concourse.bass_utils import run_bass_kernel_spmd

F32 = mybir.dt.float32
BF16 = mybir.dt.bfloat16
I16 = mybir.dt.int16

N, E, D = 50000, 800000, 256
C = 8                 # cores
SH = N // C           # 6250 real rows per shard
T = 49                # dst tiles per core
SHP = T * 128         # 6272 padded rows per shard
NP = C * SHP          # 50176 padded global rows
HALF = NP // 2        # 25088 (= shards of cores 0-3) -> table A / table B
TA = 24               # tiles 0..23 -> collective A (3072 rows/core)
TB = T - TA           # tiles 24..48 -> collective B (3200 rows/core)


def _host_prep(x, edge_index, edge_attr):
    """Pure index/layout preprocessing (no float math on values)."""
    src = np.asarray(edge_index[0], dtype=np.int64)
    dst = np.asarray(edge_index[1], dtype=np.int64)
    attr = np.asarray(edge_attr, dtype=np.float32).reshape(-1)

    owner_d = dst // SH
    dl = dst - owner_d * SH              # 0..6249
    tl = dl // 128                       # dst tile 0..48
    dcol = dl % 128
    owner_s = src // SH
    sl = src - owner_s * SH              # local src row 0..6249
    # quad: src tile-half (A: sl<3072, B: sl>=3072) x src core-half
    in_b = (sl >= TA * 128).astype(np.int64)
    in_hi = (owner_s >= 4).astype(np.int64)
    quad = in_b * 2 + in_hi
    # row in its quad gather table
    rowA = owner_s % 4 * (TA * 128) + sl
    rowB = owner_s % 4 * (TB * 128) + (sl - TA * 128)
    grow = np.where(in_b == 1, rowB, rowA)

    seg = (owner_d * T + tl) * 4 + quad
    order = np.argsort(seg, kind="stable")
    seg_sorted = seg[order]
    counts_e = np.bincount(seg_sorted, minlength=C * T * 4)
    seg_starts = np.concatenate([[0], np.cumsum(counts_e)[:-1]])
    rank_in_seg = np.arange(E) - seg_starts[seg_sorted]

    nq = counts_e.reshape(C, T, 4)
    Bq = [int(np.max((nq[:, :, q] + 127) // 128)) for q in range(4)]
    B = sum(Bq)
    e_pad = T * B * 128
    qoff = np.concatenate([[0], np.cumsum(Bq)[:-1]]) * 128   # slot offset per quad

    t_sorted = (seg_sorted // 4) % T
    q_sorted = seg_sorted % 4
    core_sorted = seg_sorted // (T * 4)
    slot = t_sorted * (B * 128) + qoff[q_sorted] + rank_in_seg

    gidx_all = np.full((C, e_pad), -1, dtype=np.int64)
    dcol_all = np.full((C, e_pad), -1.0, dtype=np.float32)
    attr_all = np.zeros((C, e_pad), dtype=np.float32)
    gidx_all[core_sorted, slot] = grow[order]
    dcol_all[core_sorted, slot] = dcol[order].astype(np.float32)
    attr_all[core_sorted, slot] = attr[order]

    # per-call valid counts [C, 4T]
    counts = np.zeros((C, 4 * T), dtype=np.int32)
    for c in range(C):
        for t in range(T):
            for q in range(4):
                a = t * B * 128 + qoff[q]
                n = Bq[q] * 128
                v = int((gidx_all[c, a:a + n] >= 0).sum())
                if v == 0:
                    gidx_all[c, a] = 0
                    v = 1
                counts[c, 4 * t + q] = v

    i = np.arange(e_pad)
    idx_tiles = []
    for c in range(C):
        t16 = np.zeros((16, e_pad // 16), dtype=np.int16)
        t16[i % 16, i // 16] = gidx_all[c].astype(np.int16)
        idx_tiles.append(np.tile(t16, (8, 1)))

    attr_tm = [attr_all[c].reshape(-1, 128).T.copy() for c in range(C)]
    s01 = np.zeros((C, T, 128, B * 128), dtype=ml_dtypes.bfloat16)
    cc_i, sl_i = np.nonzero(dcol_all >= 0)
    t_i = sl_i // (B * 128)
    r_i = sl_i % 128
    b_i = (sl_i // 128) % B
    col_i = b_i * 128 + dcol_all[cc_i, sl_i].astype(np.int64)
    s01[cc_i, t_i, r_i, col_i] = 1.0
    s01_tiles = [np.ascontiguousarray(s01[c]) for c in range(C)]

    xs = []
    xf = np.asarray(x, dtype=np.float32)
    for c in range(C):
        pad = np.zeros((SHP, D), dtype=np.float32)
        pad[:SH] = xf[c * SH:(c + 1) * SH]
        xs.append(pad)
    return xs, idx_tiles, s01_tiles, attr_tm, counts, Bq, B


def _build(Bq, B):
    e_pad = T * B * 128
    NBLK = T * B
    qoff16 = [0]
    for q in range(3):
        qoff16.append(qoff16[-1] + Bq[q] * 8)

    nc = bacc.Bacc("TRN2", target_bir_lowering=False, debug=False,
                   num_devices=C, num_swdge_queues=4)

    x_d = nc.declare_dram_parameter("x", [SHP, D], F32, isOutput=False)
    idx_d = nc.declare_dram_parameter("idx", [128, e_pad // 16], I16, isOutput=False)
    s01_d = nc.declare_dram_parameter("s01", [T, 128, B * 128], BF16, isOutput=False)
    attr_d = nc.declare_dram_parameter("attr", [128, NBLK], F32, isOutput=False)
    cnt_d = nc.declare_dram_parameter("cnt", [1, 4 * T], mybir.dt.int32, isOutput=False)
    W_d = [nc.declare_dram_parameter(f"W{l+1}", [D, D], F32, isOutput=False)
           for l in range(3)]
    b_d = [nc.declare_dram_parameter(f"b{l+1}", [128, D], F32, isOutput=False)
           for l in range(3)]
    mw1_d = nc.declare_dram_parameter("mw1", [128, 8], F32, isOutput=False)
    mb1_d = nc.declare_dram_parameter("mb1", [128, 8], F32, isOutput=False)
    mw2_d = nc.declare_dram_parameter("mw2", [128, 8], F32, isOutput=False)
    mb2_d = nc.declare_dram_parameter("mb2", [128, 1], F32, isOutput=False)
    ident_d = nc.declare_dram_parameter("ident", [128, 128], BF16, isOutput=False)
    out_d = nc.declare_dram_parameter("out", [128, T, D], F32, isOutput=True)

    s_ew_d = nc.dram_tensor("s_ew", [T, 128, B * 128], BF16)
    agi_A = [nc.dram_tensor(f"agiA{l}", [TA * 128, D], BF16) for l in range(3)]
    agi_B = [nc.dram_tensor(f"agiB{l}", [TB * 128, D], BF16) for l in range(3)]
    ago_A = [nc.dram_tensor(f"agoA{l}", [C * TA * 128, D], BF16,
                            addr_space="Shared") for l in range(3)]
    ago_B = [nc.dram_tensor(f"agoB{l}", [C * TB * 128, D], BF16,
                            addr_space="Shared") for l in range(3)]
    HA = 4 * TA * 128     # 12288 rows per half of table A
    HB = 4 * TB * 128     # 12800

    AL = mybir.AluOpType

    import contextlib
    rstack = contextlib.ExitStack()
    with tile.TileContext(nc) as tc:
        with (
            tc.tile_pool(name="res", bufs=1) as res,
            tc.tile_pool(name="work", bufs=3) as work,
            tc.tile_pool(name="gath", bufs=4) as gath,
            tc.tile_pool(name="ppool", bufs=2, space="PSUM") as ppool,
            tc.tile_pool(name="ptr", bufs=2, space="PSUM") as ptr,
            tc.tile_pool(name="pagg", bufs=3, space="PSUM") as pagg,
        ):
            x_res = res.tile([128, T, D], BF16, tag="x_res")
            aggA = res.tile([128, T, D], BF16, tag="aggA")
            idx_r = res.tile([128, e_pad // 16], I16, tag="idx")
            ew_r = res.tile([128, NBLK], F32, tag="ew")
            hb16 = res.tile([128, T, D], BF16, tag="hb16")
            dinv_r = res.tile([128, T], F32, tag="dinv")
            ident_r = res.tile([128, 128], BF16, tag="ident")
            ones_r = res.tile([128, 1], BF16, tag="ones")
            Wt = [res.tile([128, 2, D], BF16, name=f"Wt{l}", tag=f"W{l}") for l in range(3)]
            bt = [res.tile([128, D], F32, name=f"bt{l}", tag=f"b{l}") for l in range(3)]
            cnt_r = res.tile([1, 4 * T], mybir.dt.int32, tag="cnt")
            mw1_r = res.tile([128, 8], F32, tag="mw1")
            mb1_r = res.tile([128, 8], F32, tag="mb1")
            mw2_r = res.tile([128, 8], F32, tag="mw2")
            mb2_r = res.tile([128, 1], F32, tag="mb2")

            nc.sync.dma_start(idx_r[:], idx_d.ap())
            nc.sync.dma_start(cnt_r[:], cnt_d.ap())
            rvs = [rstack.enter_context(nc.gpsimd.register(f"rv{k}"))
                   for k in range(4)]
            GB = 4
            warm = []
            for w in range(GB):
                for q in range(4):
                    gw = gath.tile([128, Bq[q], D], BF16, tag=f"g{q}", bufs=GB,
                                   name=f"warm{q}_{w}")
                    nc.vector.memset(gw[:], 0.0)
            nc.sync.dma_start(ident_r[:], ident_d.ap())
            nc.sync.dma_start(mw1_r[:], mw1_d.ap())
            nc.sync.dma_start(mb1_r[:], mb1_d.ap())
            nc.sync.dma_start(mw2_r[:], mw2_d.ap())
            nc.sync.dma_start(mb2_r[:], mb2_d.ap())
            for l in range(3):
                nc.gpsimd.dma_start(
                    Wt[l][:], W_d[l].ap().rearrange("(k p) o -> p k o", p=128))
                nc.sync.dma_start(bt[l][:], b_d[l].ap())
            nc.gpsimd.dma_start(
                x_res[:], x_d.ap().rearrange("(t p) d -> p t d", p=128))
            nc.gpsimd.memset(ones_r[:], 1.0)

            # ---- edge MLP ----
            attr_r = work.tile([128, NBLK], F32, tag="attr", bufs=1)
            nc.sync.dma_start(attr_r[:], attr_d.ap())
            acc = None
            for j in range(8):
                tj = work.tile([128, NBLK], F32, tag="mlptmp", bufs=2)
                nc.scalar.activation(tj[:], attr_r[:],
                                     mybir.ActivationFunctionType.Relu,
                                     bias=mb1_r[:, j:j + 1],
                                     scale=mw1_r[:, j:j + 1])
                nacc = work.tile([128, NBLK], F32, tag="mlpacc", bufs=2,
                                 name=f"acc{j}")
                if j == 0:
                    nc.vector.tensor_scalar_mul(nacc[:], tj[:], mw2_r[:, j:j + 1])
                else:
                    nc.vector.scalar_tensor_tensor(
                        nacc[:], tj[:], mw2_r[:, j:j + 1], acc[:],
                        op0=AL.mult, op1=AL.add)
                acc = nacc
            nc.scalar.activation(ew_r[:], acc[:],
                                 mybir.ActivationFunctionType.Sigmoid,
                                 bias=mb2_r[:, 0:1])

            # ---- degree pass + S_ew build ----
            for t in range(T):
                s01_t = gath.tile([128, B * 128], BF16, tag="sew", bufs=2)
                nc.sync.dma_start(s01_t[:], s01_d.ap()[t])
                sew_t = gath.tile([128, B * 128], BF16, tag="sewo", bufs=2)
                dp = ptr.tile([128, 1], F32, tag="degp", bufs=1)
                for b in range(B):
                    blk = t * B + b
                    if b % 2 == 0:
                        nc.vector.tensor_scalar_mul(
                            sew_t[:, b * 128:(b + 1) * 128],
                            s01_t[:, b * 128:(b + 1) * 128], ew_r[:, blk:blk + 1])
                    else:
                        nc.scalar.activation(
                            sew_t[:, b * 128:(b + 1) * 128],
                            s01_t[:, b * 128:(b + 1) * 128],
                            mybir.ActivationFunctionType.Copy,
                            bias=0.0, scale=ew_r[:, blk:blk + 1])
                    nc.tensor.matmul(dp[:], sew_t[:, b * 128:(b + 1) * 128],
                                     ones_r[:],
                                     start=(b == 0), stop=(b == B - 1))
                nc.sync.dma_start(s_ew_d.ap()[t], sew_t[:])
                degs = work.tile([128, 1], F32, tag="degs")
                nc.vector.tensor_scalar_add(degs[:], dp[:], 1.0)
                rec = work.tile([128, 1], F32, tag="rec")
                nc.vector.reciprocal(rec[:], degs[:])
                nc.scalar.sqrt(dinv_r[:, t:t + 1], rec[:])

            tblq = [None] * 4

            for l in range(3):
                # ---- phase A (tiles 0..TA-1 -> ccA; rest -> ccB) ----
                for t in range(T):
                    xt = x_res[:, t, :]
                    tp = ptr.tile([128, 2, 128], BF16, tag="tpsum")
                    nc.tensor.transpose(tp[:, 0, :], xt[:, 0:128], ident_r[:])
                    nc.tensor.transpose(tp[:, 1, :], xt[:, 128:256], ident_r[:])
                    xT = work.tile([128, 2, 128], BF16, tag="xT")
                    nc.vector.tensor_copy(xT[:, 0, :], tp[:, 0, :])
                    nc.vector.tensor_copy(xT[:, 1, :], tp[:, 1, :])
                    hp = ppool.tile([128, D], F32, tag="hpsum")
                    nc.tensor.matmul(hp[:], xT[:, 0, :], Wt[l][:, 0, :],
                                     start=True, stop=False)
                    nc.tensor.matmul(hp[:], xT[:, 1, :], Wt[l][:, 1, :],
                                     start=False, stop=True)
                    tmp = work.tile([128, D], F32, tag="phA")
                    nc.vector.tensor_add(tmp[:], hp[:], bt[l][:])
                    nc.vector.tensor_scalar_mul(hb16[:, t, :], tmp[:],
                                                dinv_r[:, t:t + 1])
                    if t == TA - 1:
                        nc.sync.dma_start(
                            agi_A[l].ap().rearrange("(t p) d -> p t d", p=128),
                            hb16[:, 0:TA, :])
                        ccA = nc.gpsimd.collective_compute(
                            "AllGather", AL.bypass,
                            replica_groups=[list(range(C))],
                            ins=[agi_A[l].ap().opt()],
                            outs=[ago_A[l].ap().opt()],
                        )
                nc.sync.dma_start(
                    agi_B[l].ap().rearrange("(t p) d -> p t d", p=128),
                    hb16[:, TA:T, :])
                ccB = nc.gpsimd.collective_compute(
                    "AllGather", AL.bypass,
                    replica_groups=[list(range(C))],
                    ins=[agi_B[l].ap().opt()],
                    outs=[ago_B[l].ap().opt()],
                )
                tblq[0] = ago_A[l].ap()[0:HA]
                tblq[1] = ago_A[l].ap()[HA:2 * HA]
                tblq[2] = ago_B[l].ap()[0:HB]
                tblq[3] = ago_B[l].ap()[HB:2 * HB]

                # ---- phase B: sweep 1 (A quads, needs only ccA) ----
                B01 = Bq[0] + Bq[1]
                for t in range(T):
                    base16 = t * B * 8
                    sewa = gath.tile([128, B01 * 128], BF16, tag="sewa",
                                     bufs=2, name=f"sewa_{l}_{t}")
                    nc.sync.dma_start(sewa[:], s_ew_d.ap()[t][:, 0:B01 * 128])
                    gq = []
                    for q in range(2):
                        g = gath.tile([128, Bq[q], D], BF16, tag=f"g{q}",
                                      bufs=GB, name=f"g{q}_{l}_{t}")
                        c0 = base16 + qoff16[q]
                        nc.gpsimd.reg_load(rvs[q],
                                           cnt_r[0:1, 4 * t + q:4 * t + q + 1])
                        nc.gpsimd.dma_gather(
                            g[:], tblq[q], idx_r[:, c0:c0 + Bq[q] * 8],
                            num_idxs=Bq[q] * 128, num_idxs_reg=rvs[q],
                            elem_size=D, single_packet=False,
                            queue_num=(2 * t) % 4 if q == 0 else (2 * t + 1) % 4)
                        gq.append(g)
                    ap_ = pagg.tile([128, D], F32, tag="aggp", name=f"apA_{l}_{t}")
                    b = 0
                    for q in range(2):
                        for bb in range(Bq[q]):
                            nc.tensor.matmul(
                                ap_[:], sewa[:, b * 128:(b + 1) * 128],
                                gq[q][:, bb, :],
                                start=(b == 0), stop=(b == B01 - 1))
                            b += 1
                    nc.vector.tensor_copy(aggA[:, t, :], ap_[:])
                # ---- phase B: sweep 2 (B quads + epilogue) ----
                for t in range(T):
                    base16 = t * B * 8
                    sewb = gath.tile([128, (B - B01) * 128], BF16, tag="sewb",
                                     bufs=2, name=f"sewb_{l}_{t}")
                    nc.sync.dma_start(sewb[:], s_ew_d.ap()[t][:, B01 * 128:B * 128])
                    gq = []
                    for q in (2, 3):
                        g = gath.tile([128, Bq[q], D], BF16, tag=f"g{q}",
                                      bufs=GB, name=f"g{q}_{l}_{t}")
                        c0 = base16 + qoff16[q]
                        nc.gpsimd.reg_load(rvs[q],
                                           cnt_r[0:1, 4 * t + q:4 * t + q + 1])
                        nc.gpsimd.dma_gather(
                            g[:], tblq[q], idx_r[:, c0:c0 + Bq[q] * 8],
                            num_idxs=Bq[q] * 128, num_idxs_reg=rvs[q],
                            elem_size=D, single_packet=False,
                            queue_num=(2 * t) % 4 if q == 2 else (2 * t + 1) % 4)
                        gq.append(g)
                    ap_ = pagg.tile([128, D], F32, tag="aggp", name=f"apB_{l}_{t}")
                    b = 0
                    for qi, q in enumerate((2, 3)):
                        for bb in range(Bq[q]):
                            nc.tensor.matmul(
                                ap_[:], sewb[:, b * 128:(b + 1) * 128],
                                gq[qi][:, bb, :],
                                start=(b == 0), stop=(b == B - B01 - 1))
                            b += 1
                    tmp0 = work.tile([128, D], F32, tag="phB0", name=f"t0_{l}_{t}")
                    nc.vector.tensor_add(tmp0[:], ap_[:], aggA[:, t, :])
                    tmp = work.tile([128, D], F32, tag="phB", name=f"tmpB_{l}_{t}")
                    nc.vector.tensor_add(tmp[:], tmp0[:], hb16[:, t, :])
                    if l == 2:
                        ob = work.tile([128, D], F32, tag="outb", bufs=3,
                                       name=f"ob_{t}")
                        nc.vector.tensor_scalar_mul(
                            ob[:], tmp[:], dinv_r[:, t:t + 1])
                        nc.sync.dma_start(out_d.ap()[:, t, :], ob[:])
                    else:
                        nc.vector.tensor_scalar(
                            x_res[:, t, :], tmp[:], dinv_r[:, t:t + 1], 0.0,
                            op0=AL.mult, op1=AL.max)

    nc.compile()
    return nc


_CACHE = {}


def kernel(x, edge_index, edge_attr, W1, b1, W2, b2, W3, b3, mw1, mb1, mw2, mb2):
    xs, idx_tiles, s01_tiles, attr_tm, counts, Bq, B = _host_prep(x, edge_index, edge_attr)


    key = tuple(Bq)
    if key not in _CACHE:
        _CACHE[key] = _build(Bq, B)
    nc = _CACHE[key]

    ident = np.eye(128, dtype=np.float32).astype(ml_dtypes.bfloat16)
    b_bc = [np.tile(np.asarray(b, np.float32)[None, :], (128, 1))
            for b in (b1, b2, b3)]
    mw1_b = np.tile(np.asarray(mw1, np.float32).reshape(1, 8), (128, 1))
    mb1_b = np.tile(np.asarray(mb1, np.float32).reshape(1, 8), (128, 1))
    mw2_b = np.tile(np.asarray(mw2, np.float32).reshape(1, 8), (128, 1))
    mb2_b = np.tile(np.asarray(mb2, np.float32).reshape(1, 1), (128, 1))
    Ws = [np.ascontiguousarray(np.asarray(w, np.float32)) for w in (W1, W2, W3)]

    in_maps = []
    for c in range(C):
        in_maps.append({
            "x": xs[c], "idx": idx_tiles[c], "s01": s01_tiles[c],
            "attr": attr_tm[c], "cnt": counts[c:c + 1].reshape(1, -1),
            "W1": Ws[0], "W2": Ws[1], "W3": Ws[2],
            "b1": b_bc[0], "b2": b_bc[1], "b3": b_bc[2],
            "mw1": mw1_b, "mb1": mb1_b, "mw2": mw2_b, "mb2": mb2_b,
            "ident": ident,
        })
    res = run_bass_kernel_spmd(nc, in_maps, core_ids=list(range(C)))
    kernel.last_result = res
    outs = []
    for c in range(C):
        o = res.results[c]["out"]            # [128, T, D]
        rows = o.transpose(1, 0, 2).reshape(SHP, D)[:SH]
        outs.append(rows)
    return np.concatenate(outs, axis=0).astype(np.float32)



# revision 15
# speedup vs baseline: 1.9834x; 1.0081x over previous
"""3-layer edge-gated GCN (PyG GCNConv-style) on 8 TRN2 NeuronCores.

Strategy (self-contained, shapes hardcoded for N=50000, E=800000, D=256):
  - Shard nodes 8 ways (6250/core, padded to 6272 = 49*128 rows).
  - Algebra: with deg[v] = sum_{dst=v} ew + 1, dinv = deg^-1/2,
      h'   = (x @ W + b) * dinv[:, None]
      out  = relu?( dinv * (SUM_{e: dst=v} ew_e * h'[src_e]  +  h'[v]) )
    which equals the reference GCN layer exactly (dinv[src] folded into h',
    dinv[dst] folded into the epilogue, self-loop = dinv^2 * h).
  - Per layer: local matmul -> AllGather h' (bf16) -> per-edge row gather
    (dma_gather, int16 idx, two 25088-row tables) -> segment-sum via TensorE
    matmuls against on-device-built one-hot*ew matrices -> fused epilogue.
  - Edges are partitioned by dst owner, grouped per 128-dst tile, split into
    lo/hi source-table halves, padded to a uniform block count so all 8 cores
    run the identical program (SPMD).
"""
import os
import sys
sys.path.insert(0, "/opt/trn_rl_repo")

import numpy as np
import ml_dtypes

import concourse.bass as bass
import concourse.tile as tile
from concourse import bacc, mybir
from concourse.bass_utils import run_bass_kernel_spmd

F32 = mybir.dt.float32
BF16 = mybir.dt.bfloat16
I16 = mybir.dt.int16

N, E, D = 50000, 800000, 256
C = 8                 # cores
SH = N // C           # 6250 real rows per shard
T = 49                # dst tiles per core
SHP = T * 128         # 6272 padded rows per shard
NP = C * SHP          # 50176 padded global rows
HALF = NP // 2        # 25088 (= shards of cores 0-3) -> table A / table B
TA = 24               # tiles 0..23 -> collective A (3072 rows/core)
TB = T - TA           # tiles 24..48 -> collective B (3200 rows/core)


def _host_prep(x, edge_index, edge_attr):
    """Pure index/layout preprocessing (no float math on values)."""
    src = np.asarray(edge_index[0], dtype=np.int64)
    dst = np.asarray(edge_index[1], dtype=np.int64)
    attr = np.asarray(edge_attr, dtype=np.float32).reshape(-1)

    owner_d = dst // SH
    dl = dst - owner_d * SH              # 0..6249
    tl = dl // 128                       # dst tile 0..48
    dcol = dl % 128
    owner_s = src // SH
    sl = src - owner_s * SH              # local src row 0..6249
    # quad: src tile-half (A: sl<3072, B: sl>=3072) x src core-half
    in_b = (sl >= TA * 128).astype(np.int64)
    in_hi = (owner_s >= 4).astype(np.int64)
    quad = in_b * 2 + in_hi
    # row in its quad gather table
    rowA = owner_s % 4 * (TA * 128) + sl
    rowB = owner_s % 4 * (TB * 128) + (sl - TA * 128)
    grow = np.where(in_b == 1, rowB, rowA)

    seg = (owner_d * T + tl) * 4 + quad
    order = np.argsort(seg, kind="stable")
    seg_sorted = seg[order]
    counts_e = np.bincount(seg_sorted, minlength=C * T * 4)
    seg_starts = np.concatenate([[0], np.cumsum(counts_e)[:-1]])
    rank_in_seg = np.arange(E) - seg_starts[seg_sorted]

    nq = counts_e.reshape(C, T, 4)
    Bq = [int(np.max((nq[:, :, q] + 127) // 128)) for q in range(4)]
    B = sum(Bq)
    e_pad = T * B * 128
    qoff = np.concatenate([[0], np.cumsum(Bq)[:-1]]) * 128   # slot offset per quad

    t_sorted = (seg_sorted // 4) % T
    q_sorted = seg_sorted % 4
    core_sorted = seg_sorted // (T * 4)
    slot = t_sorted * (B * 128) + qoff[q_sorted] + rank_in_seg

    gidx_all = np.full((C, e_pad), -1, dtype=np.int64)
    dcol_all = np.full((C, e_pad), -1.0, dtype=np.float32)
    attr_all = np.zeros((C, e_pad), dtype=np.float32)
    gidx_all[core_sorted, slot] = grow[order]
    dcol_all[core_sorted, slot] = dcol[order].astype(np.float32)
    attr_all[core_sorted, slot] = attr[order]

    # per-call valid counts [C, 4T]
    counts = np.zeros((C, 4 * T), dtype=np.int32)
    for c in range(C):
        for t in range(T):
            for q in range(4):
                a = t * B * 128 + qoff[q]
                n = Bq[q] * 128
                v = int((gidx_all[c, a:a + n] >= 0).sum())
                if v == 0:
                    gidx_all[c, a] = 0
                    v = 1
                counts[c, 4 * t + q] = v

    i = np.arange(e_pad)
    idx_tiles = []
    for c in range(C):
        t16 = np.zeros((16, e_pad // 16), dtype=np.int16)
        t16[i % 16, i // 16] = gidx_all[c].astype(np.int16)
        idx_tiles.append(np.tile(t16, (8, 1)))

    attr_tm = [attr_all[c].reshape(-1, 128).T.copy() for c in range(C)]
    s01 = np.zeros((C, T, 128, B * 128), dtype=ml_dtypes.bfloat16)
    cc_i, sl_i = np.nonzero(dcol_all >= 0)
    t_i = sl_i // (B * 128)
    r_i = sl_i % 128
    b_i = (sl_i // 128) % B
    col_i = b_i * 128 + dcol_all[cc_i, sl_i].astype(np.int64)
    s01[cc_i, t_i, r_i, col_i] = 1.0
    s01_tiles = [np.ascontiguousarray(s01[c]) for c in range(C)]

    xs = []
    xf = np.asarray(x, dtype=np.float32)
    for c in range(C):
        pad = np.zeros((SHP, D), dtype=np.float32)
        pad[:SH] = xf[c * SH:(c + 1) * SH]
        xs.append(pad)
    return xs, idx_tiles, s01_tiles, attr_tm, counts, Bq, B


def _build(Bq, B):
    e_pad = T * B * 128
    NBLK = T * B
    qoff16 = [0]
    for q in range(3):
        qoff16.append(qoff16[-1] + Bq[q] * 8)

    nc = bacc.Bacc("TRN2", target_bir_lowering=False, debug=False,
                   num_devices=C, num_swdge_queues=4)

    x_d = nc.declare_dram_parameter("x", [SHP, D], F32, isOutput=False)
    idx_d = nc.declare_dram_parameter("idx", [128, e_pad // 16], I16, isOutput=False)
    s01_d = nc.declare_dram_parameter("s01", [T, 128, B * 128], BF16, isOutput=False)
    attr_d = nc.declare_dram_parameter("attr", [128, NBLK], F32, isOutput=False)
    cnt_d = nc.declare_dram_parameter("cnt", [1, 4 * T], mybir.dt.int32, isOutput=False)
    W_d = [nc.declare_dram_parameter(f"W{l+1}", [D, D], F32, isOutput=False)
           for l in range(3)]
    b_d = [nc.declare_dram_parameter(f"b{l+1}", [128, D], F32, isOutput=False)
           for l in range(3)]
    mw1_d = nc.declare_dram_parameter("mw1", [128, 8], F32, isOutput=False)
    mb1_d = nc.declare_dram_parameter("mb1", [128, 8], F32, isOutput=False)
    mw2_d = nc.declare_dram_parameter("mw2", [128, 8], F32, isOutput=False)
    mb2_d = nc.declare_dram_parameter("mb2", [128, 1], F32, isOutput=False)
    ident_d = nc.declare_dram_parameter("ident", [128, 128], BF16, isOutput=False)
    out_d = nc.declare_dram_parameter("out", [128, T, D], F32, isOutput=True)

    s_ew_d = nc.dram_tensor("s_ew", [T, 128, B * 128], BF16)
    agi_A = [nc.dram_tensor(f"agiA{l}", [TA * 128, D], BF16) for l in range(3)]
    agi_B = [nc.dram_tensor(f"agiB{l}", [TB * 128, D], BF16) for l in range(3)]
    ago_A = [nc.dram_tensor(f"agoA{l}", [C * TA * 128, D], BF16,
                            addr_space="Shared") for l in range(3)]
    ago_B = [nc.dram_tensor(f"agoB{l}", [C * TB * 128, D], BF16,
                            addr_space="Shared") for l in range(3)]
    HA = 4 * TA * 128     # 12288 rows per half of table A
    HB = 4 * TB * 128     # 12800

    AL = mybir.AluOpType

    import contextlib
    rstack = contextlib.ExitStack()
    with tile.TileContext(nc) as tc:
        with (
            tc.tile_pool(name="res", bufs=1) as res,
            tc.tile_pool(name="work", bufs=3) as work,
            tc.tile_pool(name="gath", bufs=4) as gath,
            tc.tile_pool(name="ppool", bufs=2, space="PSUM") as ppool,
            tc.tile_pool(name="ptr", bufs=2, space="PSUM") as ptr,
            tc.tile_pool(name="pagg", bufs=3, space="PSUM") as pagg,
        ):
            x_res = res.tile([128, T, D], BF16, tag="x_res")
            aggA = res.tile([128, T, D], BF16, tag="aggA")
            idx_r = res.tile([128, e_pad // 16], I16, tag="idx")
            ew_r = res.tile([128, NBLK], F32, tag="ew")
            hb16 = res.tile([128, T, D], BF16, tag="hb16")
            dinv_r = res.tile([128, T], F32, tag="dinv")
            ident_r = res.tile([128, 128], BF16, tag="ident")
            ones_r = res.tile([128, 1], BF16, tag="ones")
            Wt = [res.tile([128, 2, D], BF16, name=f"Wt{l}", tag=f"W{l}") for l in range(3)]
            bt = [res.tile([128, D], F32, name=f"bt{l}", tag=f"b{l}") for l in range(3)]
            cnt_r = res.tile([1, 4 * T], mybir.dt.int32, tag="cnt")
            mw1_r = res.tile([128, 8], F32, tag="mw1")
            mb1_r = res.tile([128, 8], F32, tag="mb1")
            mw2_r = res.tile([128, 8], F32, tag="mw2")
            mb2_r = res.tile([128, 1], F32, tag="mb2")

            nc.sync.dma_start(idx_r[:], idx_d.ap())
            nc.sync.dma_start(cnt_r[:], cnt_d.ap())
            rvs = [rstack.enter_context(nc.gpsimd.register(f"rv{k}"))
                   for k in range(4)]
            GB = 4
            warm = []
            for w in range(GB):
                for q in range(4):
                    gw = gath.tile([128, Bq[q], D], BF16, tag=f"g{q}", bufs=GB,
                                   name=f"warm{q}_{w}")
                    nc.vector.memset(gw[:], 0.0)
            nc.sync.dma_start(ident_r[:], ident_d.ap())
            nc.sync.dma_start(mw1_r[:], mw1_d.ap())
            nc.sync.dma_start(mb1_r[:], mb1_d.ap())
            nc.sync.dma_start(mw2_r[:], mw2_d.ap())
            nc.sync.dma_start(mb2_r[:], mb2_d.ap())
            for l in range(3):
                nc.gpsimd.dma_start(
                    Wt[l][:], W_d[l].ap().rearrange("(k p) o -> p k o", p=128))
                nc.sync.dma_start(bt[l][:], b_d[l].ap())
            nc.gpsimd.dma_start(
                x_res[:], x_d.ap().rearrange("(t p) d -> p t d", p=128))
            nc.gpsimd.memset(ones_r[:], 1.0)

            # ---- edge MLP ----
            attr_r = work.tile([128, NBLK], F32, tag="attr", bufs=1)
            nc.sync.dma_start(attr_r[:], attr_d.ap())
            acc = None
            for j in range(8):
                tj = work.tile([128, NBLK], F32, tag="mlptmp", bufs=2)
                nc.scalar.activation(tj[:], attr_r[:],
                                     mybir.ActivationFunctionType.Relu,
                                     bias=mb1_r[:, j:j + 1],
                                     scale=mw1_r[:, j:j + 1])
                nacc = work.tile([128, NBLK], F32, tag="mlpacc", bufs=2,
                                 name=f"acc{j}")
                if j == 0:
                    nc.vector.tensor_scalar_mul(nacc[:], tj[:], mw2_r[:, j:j + 1])
                else:
                    nc.vector.scalar_tensor_tensor(
                        nacc[:], tj[:], mw2_r[:, j:j + 1], acc[:],
                        op0=AL.mult, op1=AL.add)
                acc = nacc
            nc.scalar.activation(ew_r[:], acc[:],
                                 mybir.ActivationFunctionType.Sigmoid,
                                 bias=mb2_r[:, 0:1])

            # ---- degree pass + S_ew build ----
            for t in range(T):
                s01_t = gath.tile([128, B * 128], BF16, tag="sew", bufs=2)
                nc.sync.dma_start(s01_t[:], s01_d.ap()[t])
                sew_t = gath.tile([128, B * 128], BF16, tag="sewo", bufs=2)
                dp = ptr.tile([128, 1], F32, tag="degp", bufs=1)
                for b in range(B):
                    blk = t * B + b
                    if b % 2 == 0:
                        nc.vector.tensor_scalar_mul(
                            sew_t[:, b * 128:(b + 1) * 128],
                            s01_t[:, b * 128:(b + 1) * 128], ew_r[:, blk:blk + 1])
                    else:
                        nc.scalar.activation(
                            sew_t[:, b * 128:(b + 1) * 128],
                            s01_t[:, b * 128:(b + 1) * 128],
                            mybir.ActivationFunctionType.Copy,
                            bias=0.0, scale=ew_r[:, blk:blk + 1])
                    nc.tensor.matmul(dp[:], sew_t[:, b * 128:(b + 1) * 128],
                                     ones_r[:],
                                     start=(b == 0), stop=(b == B - 1))
                nc.sync.dma_start(s_ew_d.ap()[t], sew_t[:])
                degs = work.tile([128, 1], F32, tag="degs")
                nc.vector.tensor_scalar_add(degs[:], dp[:], 1.0)
                rec = work.tile([128, 1], F32, tag="rec")
                nc.vector.reciprocal(rec[:], degs[:])
                nc.scalar.sqrt(dinv_r[:, t:t + 1], rec[:])

            tblq = [None] * 4

            for l in range(3):
                # ---- phase A (tiles 0..TA-1 -> ccA; rest -> ccB) ----
                for t in range(T):
                    xt = x_res[:, t, :]
                    tp = ptr.tile([128, 2, 128], BF16, tag="tpsum")
                    nc.tensor.transpose(tp[:, 0, :], xt[:, 0:128], ident_r[:])
                    nc.tensor.transpose(tp[:, 1, :], xt[:, 128:256], ident_r[:])
                    xT = work.tile([128, 2, 128], BF16, tag="xT")
                    nc.vector.tensor_copy(xT[:, 0, :], tp[:, 0, :])
                    nc.vector.tensor_copy(xT[:, 1, :], tp[:, 1, :])
                    hp = ppool.tile([128, D], F32, tag="hpsum")
                    nc.tensor.matmul(hp[:], xT[:, 0, :], Wt[l][:, 0, :],
                                     start=True, stop=False)
                    nc.tensor.matmul(hp[:], xT[:, 1, :], Wt[l][:, 1, :],
                                     start=False, stop=True)
                    tmp = work.tile([128, D], F32, tag="phA")
                    nc.vector.tensor_add(tmp[:], hp[:], bt[l][:])
                    nc.scalar.activation(hb16[:, t, :], tmp[:],
                                         mybir.ActivationFunctionType.Copy,
                                         bias=0.0, scale=dinv_r[:, t:t + 1])
                    if t == TA - 1:
                        nc.sync.dma_start(
                            agi_A[l].ap().rearrange("(t p) d -> p t d", p=128),
                            hb16[:, 0:TA, :])
                        ccA = nc.gpsimd.collective_compute(
                            "AllGather", AL.bypass,
                            replica_groups=[list(range(C))],
                            ins=[agi_A[l].ap().opt()],
                            outs=[ago_A[l].ap().opt()],
                        )
                nc.sync.dma_start(
                    agi_B[l].ap().rearrange("(t p) d -> p t d", p=128),
                    hb16[:, TA:T, :])
                ccB = nc.gpsimd.collective_compute(
                    "AllGather", AL.bypass,
                    replica_groups=[list(range(C))],
                    ins=[agi_B[l].ap().opt()],
                    outs=[ago_B[l].ap().opt()],
                )
                tblq[0] = ago_A[l].ap()[0:HA]
                tblq[1] = ago_A[l].ap()[HA:2 * HA]
                tblq[2] = ago_B[l].ap()[0:HB]
                tblq[3] = ago_B[l].ap()[HB:2 * HB]

                # ---- phase B: sweep 1 (A quads, needs only ccA) ----
                B01 = Bq[0] + Bq[1]
                for t in range(T):
                    base16 = t * B * 8
                    sewa = gath.tile([128, B01 * 128], BF16, tag="sewa",
                                     bufs=2, name=f"sewa_{l}_{t}")
                    nc.sync.dma_start(sewa[:], s_ew_d.ap()[t][:, 0:B01 * 128])
                    gq = []
                    for q in range(2):
                        g = gath.tile([128, Bq[q], D], BF16, tag=f"g{q}",
                                      bufs=GB, name=f"g{q}_{l}_{t}")
                        c0 = base16 + qoff16[q]
                        nc.gpsimd.reg_load(rvs[q],
                                           cnt_r[0:1, 4 * t + q:4 * t + q + 1])
                        nc.gpsimd.dma_gather(
                            g[:], tblq[q], idx_r[:, c0:c0 + Bq[q] * 8],
                            num_idxs=Bq[q] * 128, num_idxs_reg=rvs[q],
                            elem_size=D, single_packet=False,
                            queue_num=(2 * t) % 4 if q == 0 else (2 * t + 1) % 4)
                        gq.append(g)
                    ap_ = pagg.tile([128, D], F32, tag="aggp", name=f"apA_{l}_{t}")
                    b = 0
                    for q in range(2):
                        for bb in range(Bq[q]):
                            nc.tensor.matmul(
                                ap_[:], sewa[:, b * 128:(b + 1) * 128],
                                gq[q][:, bb, :],
                                start=(b == 0), stop=(b == B01 - 1))
                            b += 1
                    nc.vector.tensor_copy(aggA[:, t, :], ap_[:])
                # ---- phase B: sweep 2 (B quads + epilogue) ----
                for t in range(T):
                    base16 = t * B * 8
                    sewb = gath.tile([128, (B - B01) * 128], BF16, tag="sewb",
                                     bufs=2, name=f"sewb_{l}_{t}")
                    nc.sync.dma_start(sewb[:], s_ew_d.ap()[t][:, B01 * 128:B * 128])
                    gq = []
                    for q in (2, 3):
                        g = gath.tile([128, Bq[q], D], BF16, tag=f"g{q}",
                                      bufs=GB, name=f"g{q}_{l}_{t}")
                        c0 = base16 + qoff16[q]
                        nc.gpsimd.reg_load(rvs[q],
                                           cnt_r[0:1, 4 * t + q:4 * t + q + 1])
                        nc.gpsimd.dma_gather(
                            g[:], tblq[q], idx_r[:, c0:c0 + Bq[q] * 8],
                            num_idxs=Bq[q] * 128, num_idxs_reg=rvs[q],
                            elem_size=D, single_packet=False,
                            queue_num=(2 * t) % 4 if q == 2 else (2 * t + 1) % 4)
                        gq.append(g)
                    ap_ = pagg.tile([128, D], F32, tag="aggp", name=f"apB_{l}_{t}")
                    b = 0
                    for qi, q in enumerate((2, 3)):
                        for bb in range(Bq[q]):
                            nc.tensor.matmul(
                                ap_[:], sewb[:, b * 128:(b + 1) * 128],
                                gq[qi][:, bb, :],
                                start=(b == 0), stop=(b == B - B01 - 1))
                            b += 1
                    tmp0 = work.tile([128, D], F32, tag="phB0", name=f"t0_{l}_{t}")
                    nc.vector.tensor_add(tmp0[:], ap_[:], aggA[:, t, :])
                    tmp = work.tile([128, D], F32, tag="phB", name=f"tmpB_{l}_{t}")
                    nc.vector.tensor_add(tmp[:], tmp0[:], hb16[:, t, :])
                    if l == 2:
                        ob = work.tile([128, D], F32, tag="outb", bufs=3,
                                       name=f"ob_{t}")
                        nc.scalar.activation(
                            ob[:], tmp[:], mybir.ActivationFunctionType.Copy,
                            bias=0.0, scale=dinv_r[:, t:t + 1])
                        nc.sync.dma_start(out_d.ap()[:, t, :], ob[:])
                    else:
                        nc.scalar.activation(
                            x_res[:, t, :], tmp[:],
                            mybir.ActivationFunctionType.Relu,
                            scale=dinv_r[:, t:t + 1])

    nc.compile()
    return nc


_CACHE = {}


def kernel(x, edge_index, edge_attr, W1, b1, W2, b2, W3, b3, mw1, mb1, mw2, mb2):
    xs, idx_tiles, s01_tiles, attr_tm, counts, Bq, B = _host_prep(x, edge_index, edge_attr)


    key = tuple(Bq)
    if key not in _CACHE:
        _CACHE[key] = _build(Bq, B)
    nc = _CACHE[key]

    ident = np.eye(128, dtype=np.float32).astype(ml_dtypes.bfloat16)
    b_bc = [np.tile(np.asarray(b, np.float32)[None, :], (128, 1))
            for b in (b1, b2, b3)]
    mw1_b = np.tile(np.asarray(mw1, np.float32).reshape(1, 8), (128, 1))
    mb1_b = np.tile(np.asarray(mb1, np.float32).reshape(1, 8), (128, 1))
    mw2_b = np.tile(np.asarray(mw2, np.float32).reshape(1, 8), (128, 1))
    mb2_b = np.tile(np.asarray(mb2, np.float32).reshape(1, 1), (128, 1))
    Ws = [np.ascontiguousarray(np.asarray(w, np.float32)) for w in (W1, W2, W3)]

    in_maps = []
    for c in range(C):
        in_maps.append({
            "x": xs[c], "idx": idx_tiles[c], "s01": s01_tiles[c],
            "attr": attr_tm[c], "cnt": counts[c:c + 1].reshape(1, -1),
            "W1": Ws[0], "W2": Ws[1], "W3": Ws[2],
            "b1": b_bc[0], "b2": b_bc[1], "b3": b_bc[2],
            "mw1": mw1_b, "mb1": mb1_b, "mw2": mw2_b, "mb2": mb2_b,
            "ident": ident,
        })
    res = run_bass_kernel_spmd(nc, in_maps, core_ids=list(range(C)))
    kernel.last_result = res
    outs = []
    for c in range(C):
        o = res.results[c]["out"]            # [128, T, D]
        rows = o.transpose(1, 0, 2).reshape(SHP, D)[:SH]
        outs.append(rows)
    return np.concatenate(outs, axis=0).astype(np.float32)

